# revision 7
# baseline (speedup 1.0000x reference)
"""Pairwise-interaction kernel for Trainium2 (raw Bass), 8-core SPMD.

Computes out[b, p, :] = x[b, i(p), :] * x[b, j(p), :] for all pairs
(i < j) of the F=26 feature rows, p ordered row-major (i outer, j inner).

Sharding: data-parallel over the batch dim (16384 -> 8 x 2048), no
cross-core communication. Per core: 16 tiles of 128 samples on SBUF
partitions. For each tile the "i" row is broadcast (stride-0 AP)
against the contiguous tail x[:, i+1:] with tensor_tensor multiplies,
writing a compact [128, 10400] output tile stored in chunked DMAs.

The kernel is HBM/SDMA-bound (output is 12.5x the input), so all
device traffic is bf16: the host converts the f32 input to bf16 (RN),
the DVE multiplies bf16*bf16 -> bf16 (2x perf mode), and the host
upcasts the bf16 output back to f32. Worst-case per-element relative
error is ~3*2^-8 = 1.2%, inside the 2e-2 gate. bf16 halves HBM traffic
vs f32: 92 MB -> 46 MB per core.

Profiling-driven structure (per-SDMA-engine descriptor streaming is
the measured bottleneck: all 16 engines run at ~101% duty):
  - the whole 3.4 MB input is resident in SBUF (XB=NT) and loaded by 4
    ungated DMAs -- (tile 0), (tile 1), (tiles 2-7), (tiles 8-15) --
    so load traffic lands in the first ~17us at read rate;
  - tiles are fused in pairs (adjacent SBUF slots, [2, nrep, D] APs):
    one TT covers both tiles, halving DVE instruction overhead so
    production (~410 GB/s) always outruns the stores;
  - store DMAs use 64-pair regions (4096 B per partition = exactly one
    full SDMA packet per descriptor, avoiding runt-packet overhead),
    decoupled from the TT i-block chunks via a completion cover map;
  - YB=4 output buffers decouple the store backlog from production.

Raw-Bass sync scheme (every instruction carries at most ONE semaphore
wait — the ISA allows exactly one wait slot per instruction):
  sem_tt  +1 by the last TT of each work unit (i-block chunk)
  sem_ld  +16 per load DMA (scalar/ACT HWDGE ring)
  sem_st  +16 per store DMA (sync/SP HWDGE ring)
  vector  waits sem_ld (tile loads) and sem_st (anti-overwrite, YB
          deep) as standalone ops, then runs the unit's TTs wait-free
  stores  wait sem_tt >= cover unit of their last pair
"""

import numpy as np
import ml_dtypes

import concourse.bass as bass
from concourse import mybir
from concourse.bass_utils import run_bass_kernel_spmd

B, F, D = 16384, 26, 32
NCORES = 8
BC = B // NCORES           # 2048 samples per core
P = 128                    # SBUF partitions per tile
NT = BC // P               # 16 tiles per core
FD = F * D                 # 832
NPAIR = F * (F - 1) // 2   # 325
OD = NPAIR * D             # 10400

XB = NT                    # input tile buffers: whole input resident
YB = 4                     # output tile buffers

# Load DMA groups (tile ranges) and the sem_ld level reached when each
# group has landed (loads complete in issue order, +16 per DMA).
LOAD_GROUPS = [(0, 1), (1, 2), (2, 8), (8, 16)]
LOAD_LEVEL = {}
for _g, (_lo, _hi) in enumerate(LOAD_GROUPS):
    for _t in range(_lo, _hi):
        LOAD_LEVEL[_t] = 16 * (_g + 1)

# TT work-unit chunks as (i_lo, i_hi) ranges over the i-blocks.
CHUNKS_DEFAULT = [(0, 3), (3, 7), (7, 12), (12, 25)]
CHUNKS_T0 = [(0, 1), (1, 3), (3, 7), (7, 12), (12, 25)]

# Store regions in pairs: 64-pair regions = 4096 B per partition, one
# full SDMA packet per descriptor. Final runt region is 5 pairs.
STORE_REGIONS = [(0, 64), (64, 128), (128, 192), (192, 256),
                 (256, 320), (320, 325)]


def _chunk_pair_off(i_lo):
    # first output pair index for block i = i_lo
    return sum(F - 1 - i for i in range(i_lo))


def _build_units():
    """TT work units: (tiles, (i_lo, i_hi)). Tiles 0 and 1 per-tile
    (early store release while loads stream); pairs 1..7 fused."""
    units = []
    for r in CHUNKS_T0:
        units.append(((0,), r))
    for r in CHUNKS_DEFAULT:
        units.append(((1,), r))
    for u in range(1, NT // 2):
        tiles = (2 * u, 2 * u + 1)
        for r in CHUNKS_DEFAULT:
            units.append((tiles, r))
    return units


UNITS = _build_units()

# cover[t][p_hi] -> 1-based unit index after which pairs [0, p_hi) of
# tile t are fully computed.
_done_pairs = {t: 0 for t in range(NT)}
_cover = {t: {} for t in range(NT)}
for _k, (_tiles, (_ilo, _ihi)) in enumerate(UNITS):
    for _t in _tiles:
        _done_pairs[_t] = _chunk_pair_off(_ihi)
        _cover[_t][_done_pairs[_t]] = _k + 1


def _cover_unit(t, p_hi):
    """Smallest unit index covering pairs [0, p_hi) of tile t."""
    best = None
    for done, k in sorted(_cover[t].items()):
        if done >= p_hi:
            best = k
            break
    assert best is not None, (t, p_hi)
    return best


# store_idx[(t, region)]: 1-based issue index of the store DMA
store_idx = {}
_si = 0
for _t in range(NT):
    pass
# stores are issued region-major per tile in unit-completion order:
# interleave stores so each fires as soon as its cover unit completes.
_store_plan = []  # (cover_k, t, region)
for _t in range(NT):
    for _r in STORE_REGIONS:
        _store_plan.append((_cover_unit(_t, _r[1]), _t, _r))
# issue order: by cover unit, then tile, then region
_store_plan.sort(key=lambda e: (e[0], e[1], e[2][0]))
for _ck, _t, _r in _store_plan:
    _si += 1
    store_idx[(_t, _r)] = _si


def _dep_store(t_prev, p_lo, p_hi):
    """Latest store index of t_prev whose pair-range overlaps."""
    dep = 0
    for (rlo, rhi) in STORE_REGIONS:
        if not (rhi <= p_lo or p_hi <= rlo):
            dep = max(dep, store_idx[(t_prev, (rlo, rhi))])
    return dep


DT = mybir.dt.bfloat16
NPDT = ml_dtypes.bfloat16

_nc_cache = None


def _build_nc():
    nc = bass.Bass()
    x = nc.declare_dram_parameter("x", [BC, FD], DT, isOutput=False)
    y = nc.declare_dram_parameter("y", [BC, OD], DT, isOutput=True)
    # [P, NT, FD] view: element (p, n, m) = x[n*P + p, m]
    xpn = x[:].rearrange("(n p) m -> p n m", p=P)
    yv = y[:].rearrange("(n p) m -> n p m", p=P)

    with (
        nc.sbuf_tensor([P, XB * FD], DT) as xbuf,
        nc.sbuf_tensor([P, YB * OD], DT) as ybuf,
        nc.semaphore("sem_ld") as sem_ld,
        nc.semaphore("sem_st") as sem_st,
        nc.semaphore("sem_tt") as sem_tt,
        nc.Block() as blk,
    ):
        xts = [xbuf[:, b * FD : (b + 1) * FD] for b in range(XB)]
        yts = [ybuf[:, b * OD : (b + 1) * OD] for b in range(YB)]

        @blk.scalar
        def _(scalar):
            for lo, hi in LOAD_GROUPS:
                dst = xbuf[:, lo * FD : hi * FD].rearrange(
                    "p (n m) -> p n m", m=FD
                )
                ld = scalar.dma_start(dst, xpn[:, lo:hi, :])
                ld.then_inc(sem_ld, 16)

        @blk.sync
        def _(sync):
            for ck, t, (p_lo, p_hi) in _store_plan:
                st = sync.dma_start(
                    yv[t][:, p_lo * D : p_hi * D],
                    yts[t % YB][:, p_lo * D : p_hi * D],
                )
                st._wait_ge(sem_tt, ck)
                st.then_inc(sem_st, 16)

        @blk.vector
        def _(v):
            seen_tiles = set()
            for k, (tiles, rng) in enumerate(UNITS):
                if tiles[-1] not in seen_tiles:
                    v.wait_ge(
                        sem_ld, max(LOAD_LEVEL[t] for t in tiles)
                    )
                    seen_tiles.update(tiles)
                i_lo, i_hi = rng
                p_lo = _chunk_pair_off(i_lo)
                p_hi = _chunk_pair_off(i_hi)
                # Anti-overwrite: the ybuf region of rng in slot t%YB is
                # free once tile t-YB's overlapping stores completed.
                dep = 0
                for t in tiles:
                    if t >= YB:
                        dep = max(dep, _dep_store(t - YB, p_lo, p_hi))
                if dep:
                    v.wait_ge(sem_st, 16 * dep)

                if len(tiles) == 1:
                    t = tiles[0]
                    xt = xts[t % XB]
                    yt = yts[t % YB]
                    off = p_lo
                    for i in range(i_lo, i_hi):
                        nrep = F - 1 - i
                        in0 = (
                            xt[:, i * D : (i + 1) * D]
                            .unsqueeze(1)
                            .broadcast_to([P, nrep, D])
                        )
                        in1 = xt[:, (i + 1) * D : FD].rearrange(
                            "p (r d) -> p r d", d=D
                        )
                        outap = yt[:, off * D : (off + nrep) * D].rearrange(
                            "p (r d) -> p r d", d=D
                        )
                        tt = nc.vector.tensor_mul(outap, in0, in1)
                        off += nrep
                else:
                    # Fused pair: tiles occupy adjacent x/y slots, so a
                    # [P, 2, nrep, D] AP covers both with one TT per i.
                    t0 = tiles[0]
                    xs = t0 % XB
                    ys = t0 % YB
                    xb2 = xbuf[:, xs * FD : (xs + 2) * FD].rearrange(
                        "p (s m) -> p s m", s=2
                    )
                    yb2 = ybuf[:, ys * OD : (ys + 2) * OD].rearrange(
                        "p (s m) -> p s m", s=2
                    )
                    off = p_lo
                    for i in range(i_lo, i_hi):
                        nrep = F - 1 - i
                        in0 = (
                            xb2[:, :, i * D : (i + 1) * D]
                            .unsqueeze(2)
                            .broadcast_to([P, 2, nrep, D])
                        )
                        in1 = xb2[:, :, (i + 1) * D : FD].rearrange(
                            "p s (r d) -> p s r d", d=D
                        )
                        outap = yb2[
                            :, :, off * D : (off + nrep) * D
                        ].rearrange("p s (r d) -> p s r d", d=D)
                        tt = nc.vector.tensor_mul(outap, in0, in1)
                        off += nrep
                tt.then_inc(sem_tt, 1)

    return nc


def make_in_maps(inputs):
    """f32 [B, F, D] -> per-core bf16 shard maps (host-side RN rounding)."""
    x = (
        np.ascontiguousarray(np.asarray(inputs, dtype=np.float32))
        .reshape(B, FD)
        .astype(NPDT)
    )
    shards = x.reshape(NCORES, BC, FD)
    return [{"x": shards[c]} for c in range(NCORES)]


def kernel(inputs: np.ndarray) -> np.ndarray:
    global _nc_cache
    if _nc_cache is None:
        _nc_cache = _build_nc()
    nc = _nc_cache

    in_maps = make_in_maps(inputs)
    res = run_bass_kernel_spmd(nc, in_maps, list(range(NCORES)))
    out = np.empty((B, NPAIR, D), dtype=np.float32)
    for c in range(NCORES):
        out[c * BC : (c + 1) * BC] = (
            res.results[c]["y"].reshape(BC, NPAIR, D).astype(np.float32)
        )
    return out


# revision 9
# speedup vs baseline: 1.1393x; 1.1393x over previous
"""Pairwise-interaction kernel for Trainium2 (raw Bass), 8-core SPMD.

Computes out[b, p, :] = x[b, i(p), :] * x[b, j(p), :] for all pairs
(i < j) of the F=26 feature rows, p ordered row-major (i outer, j inner).

Sharding: data-parallel over the batch dim (16384 -> 8 x 2048), no
cross-core communication. Per core: 16 tiles of 128 samples on SBUF
partitions. For each tile the "i" row is broadcast (stride-0 AP)
against the contiguous tail x[:, i+1:] with tensor_tensor multiplies,
writing a compact [128, 10400] output tile stored in chunked DMAs.

The kernel is HBM-bound (output is 12.5x the input), so all device
traffic is bf16: the host converts the f32 input to bf16 (RN), the DVE
multiplies bf16*bf16 -> bf16 (2x perf mode), and the host upcasts the
bf16 output back to f32. Worst-case per-element relative error is
~3*2^-8 = 1.2% (two input roundings + one output rounding), inside the
2e-2 gate. bf16 halves HBM traffic vs f32: 92 MB -> 46 MB per core.

Profiling-driven structure (the wall is the store stream: window start
+ 42.6 MB / ~370 GB/s):
  - the whole 3.4 MB input is resident in SBUF (XB=NT) and loaded by 4
    ungated DMAs -- (tile 0), (tile 1), (tiles 2-7), (tiles 8-15) --
    so all load traffic lands in the first ~17us instead of stealing
    store bandwidth mid-window;
  - tile 0 leads with a 25-pair mini chunk so the first store launches
    as early as possible;
  - tiles are fused in pairs (adjacent SBUF slots, [2, nrep, D] APs):
    one TT covers both tiles, halving DVE instruction overhead so
    production (~410 GB/s) always outruns the stores;
  - the last pair rotates its chunk order so a small chunk drains last;
  - YB=4 output buffers decouple the store backlog from production.

Raw-Bass sync scheme (every instruction carries at most ONE semaphore
wait — the ISA allows exactly one wait slot per instruction):
  sem_tt  +1 by the last TT of each work unit (chunk of a tile/pair)
  sem_ld  +16 per load DMA (scalar/ACT HWDGE ring)
  sem_st  +16 per store DMA (sync/SP HWDGE ring)
  vector  waits sem_ld (tile loads) and sem_st (anti-overwrite, YB
          deep) as standalone ops, then runs the unit's TTs wait-free
  stores  wait sem_tt >= unit_index + 1
"""

import numpy as np
import ml_dtypes

import concourse.bass as bass
from concourse import mybir
from concourse.bass_utils import run_bass_kernel_spmd

B, F, D = 16384, 26, 32
NCORES = 8
BC = B // NCORES           # 2048 samples per core
P = 128                    # SBUF partitions per tile
NT = BC // P               # 16 tiles per core
FD = F * D                 # 832
NPAIR = F * (F - 1) // 2   # 325
OD = NPAIR * D             # 10400

XB = NT                    # input tile buffers: whole input resident
YB = 4                     # output tile buffers

# Load DMA groups (tile ranges) and the sem_ld level reached when each
# group has landed (loads complete in issue order, +16 per DMA).
LOAD_GROUPS = [(0, 1), (1, 2), (2, 8), (8, 16)]
LOAD_LEVEL = {}
for _g, (_lo, _hi) in enumerate(LOAD_GROUPS):
    for _t in range(_lo, _hi):
        LOAD_LEVEL[_t] = 16 * (_g + 1)

# Store chunks as (i_lo, i_hi) ranges over the i-blocks.
CHUNKS_DEFAULT = [(0, 3), (3, 7), (7, 12), (12, 25)]
CHUNKS_T0 = [(0, 1), (1, 3), (3, 7), (7, 12), (12, 25)]


def _chunk_pair_off(i_lo):
    # first output pair index for block i = i_lo
    return sum(F - 1 - i for i in range(i_lo))


def _build_units():
    """Work units: (tiles, (i_lo, i_hi)). Tile 0 per-tile with a mini
    leading chunk (early first store); tile 1 per-tile; pairs 1..7
    fused; last pair rotated (small chunk last)."""
    units = []
    for r in CHUNKS_T0:
        units.append(((0,), r))
    for r in CHUNKS_DEFAULT:
        units.append(((1,), r))
    for u in range(1, NT // 2):
        tiles = (2 * u, 2 * u + 1)
        order = list(CHUNKS_DEFAULT)
        if u == NT // 2 - 1:
            order = order[1:] + order[:1]
        for r in order:
            units.append((tiles, r))
    return units


UNITS = _build_units()

# store_idx[(t, range)]: 1-based issue index of the store DMA
store_idx = {}
_si = 0
for _k, (_tiles, _r) in enumerate(UNITS):
    for _t in _tiles:
        _si += 1
        store_idx[(_t, _r)] = _si

# ranges stored per tile (for overlap-based anti-overwrite deps)
tile_ranges = {}
for (_t, _r), _idx in store_idx.items():
    tile_ranges.setdefault(_t, []).append((_r, _idx))


def _dep_store(t_prev, rng):
    """Latest store index of t_prev whose i-range overlaps rng."""
    lo, hi = rng
    dep = 0
    for (plo, phi), idx in tile_ranges[t_prev]:
        if not (phi <= lo or hi <= plo):
            dep = max(dep, idx)
    return dep


DT = mybir.dt.bfloat16
NPDT = ml_dtypes.bfloat16

_nc_cache = None


def _build_nc():
    nc = bass.Bass()
    x = nc.declare_dram_parameter("x", [BC, FD], DT, isOutput=False)
    y = nc.declare_dram_parameter("y", [BC, OD], DT, isOutput=True)
    # [P, NT, FD] view: element (p, n, m) = x[n*P + p, m]
    xpn = x[:].rearrange("(n p) m -> p n m", p=P)
    yflat = y[:].rearrange("a b -> (a b)")

    with (
        nc.sbuf_tensor([P, XB * FD], DT) as xbuf,
        nc.sbuf_tensor([P, YB * OD], DT) as ybuf,
        nc.semaphore("sem_ld") as sem_ld,
        nc.semaphore("sem_st") as sem_st,
        nc.semaphore("sem_tt") as sem_tt,
        nc.Block() as blk,
    ):
        xts = [xbuf[:, b * FD : (b + 1) * FD] for b in range(XB)]
        yts = [ybuf[:, b * OD : (b + 1) * OD] for b in range(YB)]

        @blk.scalar
        def _(scalar):
            for lo, hi in LOAD_GROUPS:
                n = hi - lo
                dst = xbuf[:, lo * FD : hi * FD].rearrange(
                    "p (n m) -> p n m", m=FD
                )
                ld = scalar.dma_start(dst, xpn[:, lo:hi, :])
                ld.then_inc(sem_ld, 16)

        @blk.sync
        def _(sync):
            for k, (tiles, (i_lo, i_hi)) in enumerate(UNITS):
                p_lo = _chunk_pair_off(i_lo)
                p_hi = _chunk_pair_off(i_hi)
                n = (p_hi - p_lo) * D
                for t in tiles:
                    base = t * P * OD + p_lo * D * P
                    dst = yflat[base : base + P * n].rearrange(
                        "(p m) -> p m", p=P
                    )
                    st = sync.dma_start(
                        dst,
                        yts[t % YB][:, p_lo * D : p_hi * D],
                    )
                    st._wait_ge(sem_tt, k + 1)
                    st.then_inc(sem_st, 16)

        @blk.vector
        def _(v):
            seen_tiles = set()
            for k, (tiles, rng) in enumerate(UNITS):
                if tiles[-1] not in seen_tiles:
                    v.wait_ge(
                        sem_ld, max(LOAD_LEVEL[t] for t in tiles)
                    )
                    seen_tiles.update(tiles)
                # Anti-overwrite: the ybuf region of rng in slot t%YB is
                # free once tile t-YB's overlapping stores completed.
                dep = 0
                for t in tiles:
                    if t >= YB:
                        dep = max(dep, _dep_store(t - YB, rng))
                if dep:
                    v.wait_ge(sem_st, 16 * dep)

                i_lo, i_hi = rng
                if len(tiles) == 1:
                    t = tiles[0]
                    xt = xts[t % XB]
                    yt = yts[t % YB]
                    off = _chunk_pair_off(i_lo)
                    for i in range(i_lo, i_hi):
                        nrep = F - 1 - i
                        in0 = (
                            xt[:, i * D : (i + 1) * D]
                            .unsqueeze(1)
                            .broadcast_to([P, nrep, D])
                        )
                        in1 = xt[:, (i + 1) * D : FD].rearrange(
                            "p (r d) -> p r d", d=D
                        )
                        outap = yt[:, off * D : (off + nrep) * D].rearrange(
                            "p (r d) -> p r d", d=D
                        )
                        tt = nc.vector.tensor_mul(outap, in0, in1)
                        off += nrep
                else:
                    # Fused pair: tiles occupy adjacent x/y slots, so a
                    # [P, 2, nrep, D] AP covers both with one TT per i.
                    t0 = tiles[0]
                    xs = t0 % XB
                    ys = t0 % YB
                    xb2 = xbuf[:, xs * FD : (xs + 2) * FD].rearrange(
                        "p (s m) -> p s m", s=2
                    )
                    yb2 = ybuf[:, ys * OD : (ys + 2) * OD].rearrange(
                        "p (s m) -> p s m", s=2
                    )
                    off = _chunk_pair_off(i_lo)
                    for i in range(i_lo, i_hi):
                        nrep = F - 1 - i
                        in0 = (
                            xb2[:, :, i * D : (i + 1) * D]
                            .unsqueeze(2)
                            .broadcast_to([P, 2, nrep, D])
                        )
                        in1 = xb2[:, :, (i + 1) * D : FD].rearrange(
                            "p s (r d) -> p s r d", d=D
                        )
                        outap = yb2[
                            :, :, off * D : (off + nrep) * D
                        ].rearrange("p s (r d) -> p s r d", d=D)
                        tt = nc.vector.tensor_mul(outap, in0, in1)
                        off += nrep
                tt.then_inc(sem_tt, 1)

    return nc


def make_in_maps(inputs):
    """f32 [B, F, D] -> per-core bf16 shard maps (host-side RN rounding)."""
    x = (
        np.ascontiguousarray(np.asarray(inputs, dtype=np.float32))
        .reshape(B, FD)
        .astype(NPDT)
    )
    shards = x.reshape(NCORES, BC, FD)
    return [{"x": shards[c]} for c in range(NCORES)]


def kernel(inputs: np.ndarray) -> np.ndarray:
    global _nc_cache
    if _nc_cache is None:
        _nc_cache = _build_nc()
    nc = _nc_cache

    in_maps = make_in_maps(inputs)
    res = run_bass_kernel_spmd(nc, in_maps, list(range(NCORES)))
    out = np.empty((B, NPAIR, D), dtype=np.float32)
    for c in range(NCORES):
        yf = res.results[c]["y"].reshape(-1)
        oc = out[c * BC : (c + 1) * BC]
        for t in range(NT):
            chunks = CHUNKS_T0 if t == 0 else CHUNKS_DEFAULT
            for (i_lo, i_hi) in chunks:
                p_lo = _chunk_pair_off(i_lo)
                p_hi = _chunk_pair_off(i_hi)
                npair = p_hi - p_lo
                base = t * P * OD + p_lo * D * P
                blk = yf[base : base + P * npair * D].reshape(P, npair, D)
                oc[t * P : (t + 1) * P, p_lo:p_hi, :] = blk.astype(np.float32)
    return out
